# revision 22
# baseline (speedup 1.0000x reference)
"""Trainium2 Bass kernel for an Attention + dense-MoE transformer layer.

Distribution: pure data-parallel over the batch dim (B=8) across 8
NeuronCores — one batch element per core, weights replicated. The dense
MoE (every token through every expert, gate-weighted sum) means compute
is identical under any sharding; DP avoids all collectives.

Per-core pipeline (S=1024 tokens, D=1024, H=16 heads, F=4096, E=8):
  rmsnorm1 (g1 folded into Wq/Wk/Wv on host) -> PE-transpose xnT
  -> QKV (q,k feature-major; v token-major)
  -> per-head scoresT = k_h^T.T@q_h^T -> exp (no max; values bounded)
  -> denom via ones-matmul (cross-partition sum, broadcast) -> av matmul
  -> scale by 1/denom -> Wo proj + residual -> rmsnorm2 (g2 folded into
  Wg/W1) -> x1nT -> gate softmax -> per-expert h=relu(x1n@W1e+b1),
  y=h@W2e, acc += g*(y+b2) -> out (accumulated in-place on x1).

Attention matmuls run in bf16. The MoE (>90% of the FLOPs) runs in
fp8e4 with DoubleRow perf mode (2 fp8 weights per PE cell -> 2x
throughput): x1n, W1, h, W2 are all fp8e4, PSUM accumulation stays f32.
W1 is host-scaled by 32 and W2 by 64 so their values sit in e4m3's
normal range (the raw weights are ~N(0,1/32)/N(0,1/64) — mostly
subnormal in e4m3); the inverse scales fold into the ACT bias-relu
(scale=1/32) and the gate coefficients (gate/64). Weights are pre-cast
on the host (ml_dtypes.float8_e4m3 is bit-exact with TRN FP8_EXP4 for
|x| <= 240) and pre-rearranged so each expert's W1/W2 loads with one
fully-contiguous 32KB-per-partition DMA. Attention weights are
host-cast to bf16.

The attention core interleaves, per k-tile, the current iteration's
score matmuls with the previous iteration's denominator and a@v
matmuls, so the in-order PE queue never head-of-line blocks behind
scores waiting on the (scalar-engine-bound) exp drains.

The residual stream lives in per-st [128,1024] tiles so the first
rmsnorm starts after one x DMA and the final per-st output DMAs fire as
the last expert's drains complete (dependency-driven overlap).
"""
import sys

if '/opt/trn_rl_repo' not in sys.path:
    sys.path.insert(0, '/opt/trn_rl_repo')

import numpy as np
import ml_dtypes

import concourse.bass as bass
import concourse.tile as tile
from concourse import bacc, mybir
from concourse.masks import make_identity
from concourse.bass_utils import run_bass_kernel_spmd

F32 = mybir.dt.float32
BF16 = mybir.dt.bfloat16
FP8 = mybir.dt.float8e4
AX = mybir.AxisListType.X
AF = mybir.ActivationFunctionType
PM = mybir.MatmulPerfMode
OP = mybir.AluOpType

B, S, D, H, F, E = 8, 1024, 1024, 16, 4096, 8
DH = D // H            # 64 head dim
ST = S // 128          # 8 token tiles
DT = D // 128          # 8 feature tiles
FT = F // 128          # 32 ffn tiles
EPS = 1e-6
SCALE = DH ** -0.5     # 0.125
NCORES = 8
CH = 512               # attention s_q chunk
NCH = S // CH          # 4
W1S = 32.0             # host scale on W1 (fp8 range)
W2S = 64.0             # host scale on W2


def build(use_b2):
    nc = bacc.Bacc("TRN2", target_bir_lowering=False)

    x = nc.declare_dram_parameter("x", [S, D], F32, isOutput=False)
    # host-rearranged: Wq_r[mt, p, kt, m] = g1[kt*128+p]*Wq[kt*128+p, mt*128+m]
    Wq = nc.declare_dram_parameter("Wq", [DT, 128, DT, 128], BF16, isOutput=False)
    Wk = nc.declare_dram_parameter("Wk", [DT, 128, DT, 128], BF16, isOutput=False)
    # host-rearranged: Wv_r[nh, p, kt, n] = g1[kt*128+p]*Wv[kt*128+p, nh*512+n]
    Wv = nc.declare_dram_parameter("Wv", [2, 128, DT, 512], BF16, isOutput=False)
    Wo = nc.declare_dram_parameter("Wo", [2, 128, DT, 512], BF16, isOutput=False)
    Wg = nc.declare_dram_parameter("Wg", [D, E], BF16, isOutput=False)
    # host-rearranged: W1p[e, p, kt, f] = g2[kt*128+p] * W1[e, kt*128+p, f] * 32
    W1 = nc.declare_dram_parameter("W1", [E, 128, DT, F], FP8, isOutput=False)
    # host-rearranged: W2p[e, p, ft, d] = W2[e, ft*128+p, d] * 64
    W2 = nc.declare_dram_parameter("W2", [E, 128, FT, D], FP8, isOutput=False)
    b1 = nc.declare_dram_parameter("b1", [E, F], F32, isOutput=False)
    b2 = nc.declare_dram_parameter("b2", [E, D], BF16, isOutput=False)
    out = nc.declare_dram_parameter("out", [S, D], F32, isOutput=True)

    with tile.TileContext(nc) as tc:
        with tc.tile_pool(name="pers", bufs=1) as pers, \
             tc.tile_pool(name="x1p", bufs=1) as x1p, \
             tc.tile_pool(name="tmp", bufs=2) as tmp, \
             tc.tile_pool(name="small", bufs=4) as small:

            # ---- persistent setup ----
            xr = x.ap().rearrange("(st p) d -> p st d", p=128)
            xst = []
            for st in range(ST):
                xt = pers.tile([128, D], F32, tag=f"x{st}")
                nc.sync.dma_start(out=xt, in_=xr[:, st, :])
                xst.append(xt)
            ident = pers.tile([128, 128], F32)
            make_identity(nc, ident)
            ones_bf = pers.tile([128, 128], BF16)
            nc.vector.memset(ones_bf, 1.0)
            ident_bf = pers.tile([128, 128], BF16)
            make_identity(nc, ident_bf)
            eps_sb = pers.tile([128, 1], F32)
            nc.vector.memset(eps_sb, EPS)
            wg_sb = pers.tile([128, DT, E], BF16)
            nc.gpsimd.dma_start(out=wg_sb, in_=Wg.ap().rearrange("(kt p) e -> p kt e", p=128))
            b1_sb = pers.tile([128, E, FT], F32)
            nc.sync.dma_start(out=b1_sb, in_=b1.ap().rearrange("e (ft p) -> p e ft", p=128))
            gate_sb = pers.tile([128, ST, E], F32)
            gsc_sb = pers.tile([128, ST, E], F32)   # gate / W2S for the y drain
            x1nT = x1p.tile([128, DT, S], BF16, tag="x1bf")
            x1n8 = x1p.tile([128, DT, S], FP8, tag="x1f8")

            def rms_chunk(st, dstT, ps_tp, f8dst=None):
                """token-major rmsnorm of residual tile st (no gain — folded
                into the consumer weights on the host), PE-transposed into
                dstT [128, DT, S] bf16 feature-major; optional fused fp8
                copy of the chunk on GpSimd."""
                xs = xst[st]
                sq = tmp.tile([128, D], F32, tag="scr")
                ss = small.tile([128, 1], F32, tag="ss")
                nc.scalar.activation(sq, xs, AF.Square, accum_out=ss)
                rstd = small.tile([128, 1], F32, tag="rstd")
                nc.scalar.activation(rstd, ss, AF.Sqrt, bias=eps_sb, scale=1.0 / D)
                rs = small.tile([128, 1], F32, tag="rs")
                nc.vector.reciprocal(rs, rstd)
                xnb = tmp.tile([128, D], BF16, tag="xnb")
                nc.scalar.activation(xnb, xs, AF.Copy, scale=rs)
                for dt_ in range(DT):
                    tp = ps_tp.tile([128, 128], BF16, tag="tp")
                    nc.tensor.transpose(
                        tp, xnb[:, dt_ * 128:(dt_ + 1) * 128], ident_bf)
                    nc.vector.tensor_copy(
                        dstT[:, dt_, st * 128:(st + 1) * 128], tp)
                if f8dst is not None:
                    nc.gpsimd.tensor_copy(
                        f8dst[:, :, st * 128:(st + 1) * 128],
                        dstT[:, :, st * 128:(st + 1) * 128])

            def rmsnorm_transpose(dstT, ps_tp):
                for st in range(ST):
                    rms_chunk(st, dstT, ps_tp)

            # ================= Scope I: attention =================
            with tc.tile_pool(name="attn", bufs=1) as attn, \
                 tc.tile_pool(name="wbigp", bufs=2) as wbigp, \
                 tc.tile_pool(name="gpool", bufs=1) as gpool:
                xnT = attn.tile([128, DT, S], BF16, tag="xT")
                qT = attn.tile([128, DT, S], BF16, tag="qT")
                kT = attn.tile([128, DT, S], BF16, tag="kT")
                v_sb = attn.tile([128, ST, D], BF16, tag="v")

                with tc.tile_pool(name="ps12", bufs=3, space="PSUM") as ps12, \
                     tc.tile_pool(name="wsa", bufs=2) as wsa:
                    # prefetch Wv halves while rmsnorm/QK run
                    wv_c = []
                    for nh in range(2):
                        wv = wbigp.tile([128, DT, 512], BF16, tag=f"wv{nh}")
                        nc.sync.dma_start(out=wv, in_=Wv[nh])
                        wv_c.append(wv)

                    rmsnorm_transpose(xnT, ps12)

                    for wh, dstT in ((Wq, qT), (Wk, kT)):
                        for mt in range(DT):
                            wbf = wsa.tile([128, DT, 128], BF16, tag="wbf")
                            nc.sync.dma_start(out=wbf, in_=wh[mt])
                            for nh in range(2):
                                ps = ps12.tile([128, 512], F32, tag="mm")
                                for kt in range(DT):
                                    nc.tensor.matmul(
                                        ps, wbf[:, kt, :],
                                        xnT[:, kt, nh * 512:(nh + 1) * 512],
                                        start=(kt == 0), stop=(kt == DT - 1))
                                nc.vector.tensor_copy(
                                    dstT[:, mt, nh * 512:(nh + 1) * 512], ps)

                    for nh in range(2):
                        for st in range(ST):
                            ps = ps12.tile([128, 512], F32, tag="mm")
                            for kt in range(DT):
                                nc.tensor.matmul(
                                    ps, xnT[:, kt, st * 128:(st + 1) * 128],
                                    wv_c[nh][:, kt, :],
                                    start=(kt == 0), stop=(kt == DT - 1))
                            nc.vector.tensor_copy(
                                v_sb[:, st, nh * 512:(nh + 1) * 512], ps)

                # ---- attention core ----
                # Head PAIRS (2t, 2t+1) share one 128-row tile of qT/kT:
                # even head in partitions 0-63 (PE row-group 0), odd in
                # 64-127 — the score pair runs concurrently on disjoint PE
                # row groups; dn/av stack the pair on PSUM col groups 0/64.
                # Per k-tile, the current iteration's score MMs interleave
                # with the previous iteration's dn/av MMs so the in-order PE
                # queue always has exp-independent work while ACT drains.
                avT = attn.tile([128, DT, S], BF16, tag="xT")  # reuses xnT slot
                with tc.tile_pool(name="ps3a", bufs=3, space="PSUM") as ps3a, \
                     tc.tile_pool(name="ps3b", bufs=1, space="PSUM") as ps3b, \
                     tc.tile_pool(name="expp", bufs=2) as expp, \
                     tc.tile_pool(name="recp", bufs=2) as recp:

                    def emit_iteration(cur, prev):
                        if prev is not None:
                            pt, pcs, pexp_e, pexp_o = prev
                            ps_dn = ps3b.tile([128, CH], F32, tag="dn")
                            ps_av = ps3b.tile([128, CH], F32, tag="av")
                        if cur is not None:
                            t, cs, exp_e, exp_o = cur
                        for kt in range(ST):
                            if cur is not None:
                                ks = slice(kt * 128, (kt + 1) * 128)
                                ps_e = ps3a.tile([128, CH], F32, tag="sce")
                                nc.tensor.matmul(
                                    ps_e, kT[0:64, t, ks], qT[0:64, t, cs],
                                    start=True, stop=True)
                                ps_o = ps3a.tile([128, CH], F32, tag="sco")
                                nc.tensor.matmul(
                                    ps_o, kT[64:128, t, ks], qT[64:128, t, cs],
                                    start=True, stop=True)
                            if prev is not None:
                                nc.tensor.matmul(
                                    ps_dn[0:64, :], ones_bf[:, 0:64],
                                    pexp_e[:, kt, :],
                                    start=(kt == 0), stop=(kt == ST - 1))
                                nc.tensor.matmul(
                                    ps_dn[64:128, :], ones_bf[:, 64:128],
                                    pexp_o[:, kt, :],
                                    start=(kt == 0), stop=(kt == ST - 1))
                                nc.tensor.matmul(
                                    ps_av[0:64, :],
                                    v_sb[:, kt, (2 * pt) * 64:(2 * pt) * 64 + 64],
                                    pexp_e[:, kt, :],
                                    start=(kt == 0), stop=(kt == ST - 1))
                                nc.tensor.matmul(
                                    ps_av[64:128, :],
                                    v_sb[:, kt, (2 * pt + 1) * 64:(2 * pt + 1) * 64 + 64],
                                    pexp_o[:, kt, :],
                                    start=(kt == 0), stop=(kt == ST - 1))
                            if cur is not None:
                                nc.scalar.activation(
                                    exp_e[:, kt, :], ps_e, AF.Exp, scale=SCALE)
                                nc.scalar.activation(
                                    exp_o[:, kt, :], ps_o, AF.Exp, scale=SCALE)
                        if prev is not None:
                            recipb = recp.tile([128, CH], F32, tag="recip")
                            nc.vector.reciprocal_approx_fast(recipb, ps_dn)
                            nc.vector.tensor_mul(avT[:, pt, pcs], ps_av, recipb)

                    prev = None
                    for t in range(H // 2):
                        for c in range(NCH):
                            cs = slice(c * CH, (c + 1) * CH)
                            exp_e = expp.tile([128, ST, CH], BF16, tag="expe")
                            exp_o = expp.tile([128, ST, CH], BF16, tag="expo")
                            cur = (t, cs, exp_e, exp_o)
                            emit_iteration(cur, prev)
                            prev = cur
                    emit_iteration(None, prev)

                # ---- Wo proj + residual, rmsnorm2, gate ----
                with tc.tile_pool(name="ps4", bufs=3, space="PSUM") as ps4, \
                     tc.tile_pool(name="ps4b", bufs=1, space="PSUM") as ps4b:
                    wo_cs = []
                    for nh in range(2):
                        wo_c = wbigp.tile([128, DT, 512], BF16, tag=f"wv{nh}")
                        nc.sync.dma_start(out=wo_c, in_=Wo[nh])
                        wo_cs.append(wo_c)
                    for st in range(ST):
                        for nh in range(2):
                            ps = ps4.tile([128, 512], F32, tag="mm")
                            for kt in range(DT):
                                nc.tensor.matmul(
                                    ps, avT[:, kt, st * 128:(st + 1) * 128],
                                    wo_cs[nh][:, kt, :],
                                    start=(kt == 0), stop=(kt == DT - 1))
                            nc.vector.tensor_add(
                                xst[st][:, nh * 512:(nh + 1) * 512],
                                xst[st][:, nh * 512:(nh + 1) * 512], ps)
                        # rmsnorm2 chunk rides right behind this st's Wo
                        # chains; fused fp8 cast feeds the MoE
                        rms_chunk(st, x1nT, ps4, f8dst=x1n8)

                    # gate = softmax(x1n @ Wg): logits computed with Wg as
                    # the (8-col) stationary -> [8, S] in PSUM, transposed
                    # back to token-major for the softmax.
                    for nh in range(2):
                        psg = ps4b.tile([8, 512], F32, tag="psg")
                        for kt in range(DT):
                            nc.tensor.matmul(
                                psg, wg_sb[:, kt, :],
                                x1nT[:, kt, nh * 512:(nh + 1) * 512],
                                start=(kt == 0), stop=(kt == DT - 1))
                        lsb = gpool.tile([8, 512], F32, tag=f"lg{nh}")
                        nc.vector.tensor_copy(lsb, psg)
                        for c4 in range(4):
                            st = nh * 4 + c4
                            tpl = ps4b.tile([128, 8], F32, tag="tpl")
                            nc.tensor.transpose(
                                tpl, lsb[:, c4 * 128:(c4 + 1) * 128], ident[:8, :8])
                            gexp = small.tile([128, E], F32, tag="gexp")
                            nc.scalar.activation(gexp, tpl, AF.Exp)
                            gsum = small.tile([128, 1], F32, tag="gsum")
                            nc.vector.reduce_sum(gsum, gexp, axis=AX)
                            grec = small.tile([128, 1], F32, tag="grec")
                            nc.vector.reciprocal(grec, gsum)
                            nc.vector.tensor_scalar_mul(gate_sb[:, st, :], gexp, grec)

                    # gsc = gate / W2S (folds the host W2 scale out)
                    nc.vector.tensor_scalar_mul(gsc_sb, gate_sb, 1.0 / W2S)

                    if use_b2:
                        # out += gate @ b2 (Sum_e g_e*b2_e, done once)
                        b2rb = gpool.tile([8, D], BF16)
                        nc.gpsimd.dma_start(out=b2rb, in_=b2.ap())
                        gateT = gpool.tile([8, ST, 128], BF16)
                        for st in range(ST):
                            tpg = ps4.tile([128, 128], F32, tag="tp")
                            nc.tensor.transpose(
                                tpg[:8, :], gate_sb[:, st, :], ident)
                            nc.vector.tensor_copy(gateT[:, st, :], tpg[:8, :])
                        for st in range(ST):
                            for nh in range(2):
                                ps = ps4.tile([128, 512], F32, tag="mm")
                                nc.tensor.matmul(
                                    ps, gateT[:, st, :],
                                    b2rb[:, nh * 512:(nh + 1) * 512],
                                    start=True, stop=True)
                                nc.vector.tensor_add(
                                    xst[st][:, nh * 512:(nh + 1) * 512],
                                    xst[st][:, nh * 512:(nh + 1) * 512], ps)

            # ================= Scope II: MoE (fp8 DoubleRow) =================
            with tc.tile_pool(name="w1pool", bufs=2) as w1pool, \
                 tc.tile_pool(name="w2pool", bufs=1) as w2pool, \
                 tc.tile_pool(name="hpool", bufs=1) as hpool, \
                 tc.tile_pool(name="ps5", bufs=4, space="PSUM") as ps5:
                for e in range(E):
                    w18 = w1pool.tile([128, DT, F], FP8, tag="w1")
                    nc.sync.dma_start(out=w18, in_=W1[e])
                    w28 = w2pool.tile([128, FT, D], FP8, tag="w2")
                    nc.sync.dma_start(out=w28, in_=W2[e])
                    h8 = hpool.tile([128, FT, S], FP8, tag="h8")

                    # h = relu(x1n @ W1e + b1e), fp8, feature-major
                    for ft in range(FT):
                        for sh in range(2):
                            shs = slice(sh * 512, (sh + 1) * 512)
                            ps_h = ps5.tile([128, 512], F32, tag="h")
                            for kp in range(DT // 2):
                                nc.tensor.matmul(
                                    ps_h,
                                    w18[:, 2 * kp:2 * kp + 2, ft * 128:(ft + 1) * 128],
                                    x1n8[:, 2 * kp:2 * kp + 2, shs],
                                    start=(kp == 0), stop=(kp == DT // 2 - 1),
                                    perf_mode=PM.DoubleRow)
                            nc.scalar.activation(
                                h8[:, ft, shs], ps_h, AF.Relu,
                                bias=b1_sb[:, e, ft:ft + 1], scale=1.0 / W1S)

                    # y = h @ W2e; x1 += (gate_e/W2S) * y  (fused on DVE)
                    for st in range(ST):
                        for nh in range(2):
                            ps_y = ps5.tile([128, 512], F32, tag="y")
                            for fp_ in range(FT // 2):
                                nc.tensor.matmul(
                                    ps_y,
                                    h8[:, 2 * fp_:2 * fp_ + 2, st * 128:(st + 1) * 128],
                                    w28[:, 2 * fp_:2 * fp_ + 2, nh * 512:(nh + 1) * 512],
                                    start=(fp_ == 0), stop=(fp_ == FT // 2 - 1),
                                    perf_mode=PM.DoubleRow)
                            xs = xst[st][:, nh * 512:(nh + 1) * 512]
                            nc.vector.scalar_tensor_tensor(
                                xs, ps_y, gsc_sb[:, st, e:e + 1], xs,
                                op0=OP.mult, op1=OP.add)

            outr = out.ap().rearrange("(st p) d -> p st d", p=128)
            for st in range(ST):
                nc.sync.dma_start(out=outr[:, st, :], in_=xst[st])

    nc.finalize()
    return nc


_CACHE = {}


def _get_nc(use_b2):
    key = f'nc{int(use_b2)}'
    if key not in _CACHE:
        _CACHE[key] = build(use_b2)
    return _CACHE[key]


def _prep_weights(inputs):
    """Host-side dtype casts, gain folding, fp8 range scaling, layout."""
    key = id(inputs['W1'])
    if _CACHE.get('wkey') == key:
        return _CACHE['w']
    f8 = ml_dtypes.float8_e4m3
    bf = ml_dtypes.bfloat16
    g1 = np.asarray(inputs['g1'], np.float32)
    g2 = np.asarray(inputs['g2'], np.float32)
    w = {}
    for k in ('Wq', 'Wk'):
        wf = (g1[:, None] * np.asarray(inputs[k], np.float32)).astype(bf)
        w[k] = np.ascontiguousarray(
            wf.reshape(DT, 128, DT, 128).transpose(2, 1, 0, 3))
    wv = (g1[:, None] * np.asarray(inputs['Wv'], np.float32)).astype(bf)
    w['Wv'] = np.ascontiguousarray(
        wv.reshape(DT, 128, 2, 512).transpose(2, 1, 0, 3))
    wo = np.asarray(inputs['Wo'], np.float32).astype(bf)
    w['Wo'] = np.ascontiguousarray(
        wo.reshape(DT, 128, 2, 512).transpose(2, 1, 0, 3))
    w['Wg'] = (g2[:, None] * np.asarray(inputs['Wg'], np.float32)).astype(bf)
    w['b2'] = np.asarray(inputs['b2'], np.float32).astype(bf)
    W1 = (g2[None, :, None] * np.asarray(inputs['W1'], np.float32) * W1S).astype(f8)
    w['W1'] = np.ascontiguousarray(
        W1.reshape(E, DT, 128, F).transpose(0, 2, 1, 3))
    W2 = (np.asarray(inputs['W2'], np.float32) * W2S).astype(f8)
    w['W2'] = np.ascontiguousarray(
        W2.reshape(E, FT, 128, D).transpose(0, 2, 1, 3))
    w['b1'] = np.ascontiguousarray(np.asarray(inputs['b1'], np.float32))
    _CACHE['w'] = w
    _CACHE['wkey'] = key
    return w


def _in_maps(inputs):
    xf = np.ascontiguousarray(np.asarray(inputs['x'], dtype=np.float32))
    assert xf.shape == (B, S, D)
    nh = inputs.get('n_heads', H)
    assert int(nh) == H, f"kernel hardcodes n_heads={H}, got {nh}"
    base = _prep_weights(inputs)
    use_b2 = bool(np.any(np.asarray(inputs['b2'])))
    return [dict(base, x=xf[i]) for i in range(NCORES)], use_b2


def kernel(**inputs):
    maps, use_b2 = _in_maps(inputs)
    nc = _get_nc(use_b2)
    res = run_bass_kernel_spmd(nc, maps, core_ids=list(range(NCORES)))
    return np.stack([res.results[i]['out'] for i in range(NCORES)], axis=0)


def kernel_profiled(**inputs):
    """Like kernel() but also returns neuron-profile exec_time_ns."""
    import tempfile
    maps, use_b2 = _in_maps(inputs)
    nc = _get_nc(use_b2)
    res = run_bass_kernel_spmd(
        nc, maps, core_ids=list(range(NCORES)),
        trace=True, tmpdir=tempfile.mkdtemp())
    outv = np.stack([res.results[i]['out'] for i in range(NCORES)], axis=0)
    return outv, res.exec_time_ns
